# revision 1
# baseline (speedup 1.0000x reference)
"""DSALite sparse-attention Trainium2 kernel.

Problem: B=1, nH=4, T=4096, hd=128 attention where the mask is derived from
8x-downsampled scores: per full row, threshold = 128th largest of the 512
downsampled (and u-scaled) scores, mask = sigmoid((s - thr) * 10 * u) * causal,
scores += (1-mask) * (-1e9), softmax, @V.

Sharding: 8 cores = 4 heads x 2 row-parities.  Core (h, r) handles head h and
query blocks {128k : k % 2 == r} (interleaved 128-row blocks, which balances
the causal work across cores).  Two Bass programs are built (one per parity,
since the causal structure is baked into static column ranges) and dispatched
concurrently on devices 0-3 (parity 0) and 4-7 (parity 1).

Structure per core (Tile-scheduled, ~2.3k instructions):
  - K^T / Q^T via PE transposes; exact-f32 downsampled-score matmul; exact
    per-row 128th-largest threshold via DVE max8 + match_replace.
  - Per 128-row block: replicate ds rows via 0/1 matmul, sigmoid via
    exp + correctly-rounded DVE reciprocal, F = fl(fl(1-m)*(-1e9)),
    A = min(F, ds-causal); fp32r QK^T restricted to the causal prefix;
    X = fl(fl(S*scale) + A) on DVE (diagonal block fixed elementwise via
    copy_predicated); ACT exp with accumulated row-sum.
  - Suffix (fully-masked) columns are never materialized: their uniform
    value C = exp(-1e9 - M) contributes C * (suffix V column-sum) via K=1
    rank-1 matmuls injected into the PV PSUM accumulator, and C * n_suffix
    is added to the softmax denominator.
  - PV: PE-transpose P chunks, fp32r V^T @ P^T accumulation (512-wide),
    transpose back, scale by 1/den, one output DMA per block.

Numerical-fidelity notes (the mask path is bucket-quantized by the f32
rounding of (1-m)*(-1e9), so it is replicated op-for-op):
  - downsampled scores use exact fp32 matmuls; the big QK^T / PV matmuls use
    fp32r (1 cyc/row at N=512, ~2^-21 relative input rounding, smooth path
    only -- S-noise there provably cannot flip mask buckets).
  - sigmoid is computed as 1/(1 + exp(-arg)) with the HW's correctly-rounded
    DVE reciprocal: the ACT Sigmoid *table* is 1 ulp low at the
    saturation-to-1.0 cliff, which flips (1-m)*(-1e9) between 0 and -59.6
    and corrupts ~half the rows; the exp+reciprocal form reproduces XLA's
    f32 saturation exactly.
  - softmax subtracts M = max over allowed F (within +-|S| of the true row
    max, which only shifts num/den by a common per-row factor).

Measured: hardware absmax error 0.0104 vs the f32 reference (rel 2.7e-3,
1 row > 1e-2 of 16384); TimelineSim cost-model estimate ~217/220 us for the
two programs (run concurrently).  The block partition KBS[] balances the
critical path between the programs, and the last two groups' PV is emitted
in two halves so it overlaps those blocks' scores/exp.
"""

import os
import numpy as np

import concourse.bass as bass
import concourse.bacc as bacc
import concourse.mybir as mybir
import concourse.tile as tile
from concourse.masks import make_identity

F32 = mybir.dt.float32
F32R = mybir.dt.float32r
ALU = mybir.AluOpType
ACTF = mybir.ActivationFunctionType
AX = mybir.AxisListType

B, NH, T, HD = 1, 4, 4096, 128
STRIDE = 8
TDS = T // STRIDE          # 512 downsampled positions
KDS = 128                  # 128th largest (k=1024 over 8x-repeated row)
NEG = -1e9
SCALE = HD ** -0.5
ZAP = -1e30                # match_replace fill, far below any score

NB = 16                    # 128-row query blocks per core
QPB = 128
NLQ = NB * QPB             # 2048 local query rows
NG = 4                     # PV groups of 4 blocks (512 q)
CH = T // 128              # 32 key/value chunks
CMW = 1008                 # sliding ds-causal const width
GORDER = [0, 1, 2, 3]      # group processing order (tunable)
# absolute 128-row block indices per program; near-equal causal work
KBS = [
    [0, 2, 4, 6, 8, 10, 12, 14, 18, 20, 22, 23, 24, 27, 29, 31],
    [1, 3, 5, 7, 9, 11, 13, 15, 16, 17, 19, 21, 25, 26, 28, 30],
]


def _consts(nc, pool):
    """Build the constant tiles (identity, diag causal, ds-causal, Rep16)."""
    ident = pool.tile([128, 128], F32, tag="ident")
    make_identity(nc, ident[:])

    # c01inv[i, j] = 1 where j > i (strictly above diagonal) else 0
    c01inv = pool.tile([128, 128], mybir.dt.int8, tag="c01inv")
    nc.gpsimd.memset(c01inv[:], 1)
    nc.gpsimd.affine_select(
        out=c01inv[:], in_=c01inv[:], pattern=[[1, 128]], base=0,
        channel_multiplier=-1, compare_op=ALU.is_gt, fill=0,
    )

    # cm3[i, jj] = 0.0 if jj <= 496 + i//8 else -1e9 (sliding ds-causal mask:
    # block with absolute index kb reads cols [496-16*kb, 496-16*kb+512)).
    # jj <= 496 + i//8  <=>  -8*jj + 3968 + i >= 0  (integer jj, i)
    cm3 = pool.tile([128, CMW], F32, tag="cm3")
    nc.gpsimd.memset(cm3[:], 0.0)
    nc.gpsimd.affine_select(
        out=cm3[:], in_=cm3[:], pattern=[[-8, CMW]], base=3968,
        channel_multiplier=1, compare_op=ALU.is_ge, fill=NEG,
    )

    # rep128[bp][rr, i] = 1.0 iff rr == 16*bp + i//8 : replicates the 16 ds
    # rows at partitions [16bp, 16bp+16) onto 128 rows (matmul lhsT).
    reps = []
    for bp in range(8):
        rep = pool.tile([128, 128], F32, tag=f"rep{bp}")
        nc.gpsimd.memset(rep[:], 1.0)
        # keep where (i - 8*rr + 128*bp) >= 0, else 0
        nc.gpsimd.affine_select(
            out=rep[:], in_=rep[:], pattern=[[1, 128]], base=128 * bp,
            channel_multiplier=-8, compare_op=ALU.is_ge, fill=0.0)
        # keep where (-i + 8*rr - 128*bp + 7) >= 0, else 0
        nc.gpsimd.affine_select(
            out=rep[:], in_=rep[:], pattern=[[-1, 128]], base=7 - 128 * bp,
            channel_multiplier=8, compare_op=ALU.is_ge, fill=0.0)
        reps.append(rep)

    identr = pool.tile([128, 128], F32R, tag="identr")
    nc.vector.tensor_copy(identr[:], ident[:])

    negc = pool.tile([128, 1], F32, tag="negc")
    nc.gpsimd.memset(negc[:], NEG)

    return ident, identr, c01inv, cm3, reps, negc


DEBUG = False


def _kernel_body(tc, r, Qd, Kd, Vd, Ud, Od, dbg=None):
    nc = tc.nc
    from contextlib import ExitStack
    with ExitStack() as ctx:
        cpool = ctx.enter_context(tc.tile_pool(name="consts", bufs=1))
        inpool = ctx.enter_context(tc.tile_pool(name="inputs", bufs=1))
        ps2pool = ctx.enter_context(tc.tile_pool(name="ps2", bufs=3, space="PSUM"))
        swppool = ctx.enter_context(tc.tile_pool(name="swp", bufs=1, space="PSUM"))
        ps1pool = ctx.enter_context(tc.tile_pool(name="ps1", bufs=1, space="PSUM"))
        ptpool = ctx.enter_context(tc.tile_pool(name="ptps", bufs=2, space="PSUM"))
        outtpool = ctx.enter_context(tc.tile_pool(name="outtps", bufs=1, space="PSUM"))
        sdspool = ctx.enter_context(tc.tile_pool(name="sds", bufs=1))
        scrpool = ctx.enter_context(tc.tile_pool(name="scr", bufs=1))
        epool = ctx.enter_context(tc.tile_pool(name="e", bufs=2))
        fpool = ctx.enter_context(tc.tile_pool(name="f", bufs=2))
        aminpool = ctx.enter_context(tc.tile_pool(name="amin", bufs=4))
        smallpool = ctx.enter_context(tc.tile_pool(name="small", bufs=NB))
        tinypool = ctx.enter_context(tc.tile_pool(name="tiny", bufs=2))
        ppool = ctx.enter_context(tc.tile_pool(name="p", bufs=5))
        ptsbpool = ctx.enter_context(tc.tile_pool(name="ptsb", bufs=3))
        outtsbpool = ctx.enter_context(tc.tile_pool(name="outtsb", bufs=2))
        tmppool = ctx.enter_context(tc.tile_pool(name="tmp", bufs=2))
        swsbpool = ctx.enter_context(tc.tile_pool(name="swsb", bufs=1))

        ident, identr, c01inv, cm3, reps, negc = _consts(nc, cpool)

        # ---- load inputs ----
        ub = inpool.tile([128, NB], F32, tag="ub")
        nc.sync.dma_start(out=ub[:], in_=Ud[:])
        usc = inpool.tile([128, NB], F32, tag="usc")
        # u_scale = 1 + clip(U, 0, 1) ; clip = min(max(u, 0), 1)
        nc.vector.tensor_scalar(usc[:], ub[:], 0.0, 1.0, op0=ALU.max, op1=ALU.min)
        nc.vector.tensor_scalar(usc[:], usc[:], 1.0, None, op0=ALU.add)

        vsb = inpool.tile([128, T], F32R, tag="vsb")  # [t_local, c, d] natural
        nc.sync.dma_start(
            out=vsb[:].rearrange("p (c d) -> p c d", d=128),
            in_=Vd.rearrange("(c p) d -> p c d", p=128),
        )

        # f32r copies feed the big QK/PV matmuls; small exact-f32 strided
        # copies (every 8th column) feed the mask path.
        kt = inpool.tile([128, T], F32R, tag="kt")    # K^T  [d, t]
        qt = inpool.tile([128, NLQ], F32R, tag="qt")  # Q^T  [d, q_local]
        kdst = inpool.tile([128, TDS], F32, tag="kdst")
        qdst = inpool.tile([128, TDS // 2], F32, tag="qdst")
        with tc.tile_pool(name="prep", bufs=3) as prep:
            for src, dst, dsdst, npieces in (
                    (Kd, kt, kdst, 8), (Qd, qt, qdst, 4)):
                s3 = src.rearrange("(c p) d -> p c d", p=128)
                for c4 in range(npieces):
                    nat = prep.tile([128, 512], F32, tag="nat")
                    nc.sync.dma_start(
                        out=nat[:].rearrange("p (c d) -> p c d", d=128),
                        in_=s3[:, 4 * c4:4 * c4 + 4, :],
                    )
                    n3 = nat[:].rearrange("p (c d) -> p c d", d=128)
                    pt = ps2pool.tile([128, 512], F32, tag="ps2")
                    for j in range(4):
                        nc.tensor.transpose(
                            pt[:, 128 * j:128 * j + 128], n3[:, j, :], ident[:])
                    nc.any.tensor_copy(dst[:, 512 * c4:512 * c4 + 512], pt[:])
                    nc.any.tensor_copy(
                        dsdst[:, 64 * c4:64 * c4 + 64],
                        pt[:].rearrange("p (c s) -> p c s", s=STRIDE)[:, :, 0])

        # ---- downsampled scores + exact per-row 128th largest ----
        sds_tiles = {}
        for t in (0, 1):
            sds = sdspool.tile([128, TDS + 1], F32, tag=f"sds{t}")
            sds_tiles[t] = sds
            ps = ps2pool.tile([128, 512], F32, tag="ps2")
            nc.tensor.matmul(ps[:], qdst[:, 128 * t:128 * t + 128], kdst[:])
            nc.scalar.mul(sds[:, 0:TDS], ps[:], SCALE)
            scr = scrpool.tile([128, TDS], F32, tag="scr")
            nc.vector.tensor_copy(scr[:], sds[:, 0:TDS])
            maxsc = scrpool.tile([128, 8], F32, tag="maxsc")
            for rnd in range(KDS // 8):
                nc.vector.max(out=maxsc[:], in_=scr[:])
                if rnd < KDS // 8 - 1:
                    nc.vector.match_replace(
                        out=scr[:], in_to_replace=maxsc[:], in_values=scr[:],
                        imm_value=ZAP,
                    )
            nc.vector.tensor_copy(sds[:, TDS:TDS + 1], maxsc[:, 7:8])
            if dbg is not None:
                nc.sync.dma_start(out=dbg[f"SDS{t}"], in_=sds[:])

        # ---- per-block processing: mask chain + scores + exp; PV per group.
        # Only the causally-allowed ds columns [0, 16*(kb+1)) are computed;
        # the tail of A is a constant -1e9.
        amins, negms, cees = {}, {}, {}
        ptiles, rsums = {}, {}

        def mask_chain(b):
            kb = KBS[r][b]
            nd = 16 * (kb + 1)
            tt, pp = divmod(b, 8)
            sds = sds_tiles[tt]
            ps = ps2pool.tile([128, 512], F32, tag="ps2")
            nc.tensor.matmul(ps[:, 0:nd], reps[pp][:], sds[:, 0:nd])
            ps1 = ps1pool.tile([128, 1], F32, tag="ps1")
            nc.tensor.matmul(ps1[:], reps[pp][:], sds[:, TDS:TDS + 1])
            thru = tinypool.tile([128, 1], F32, tag="thru")
            nc.scalar.mul(thru[:], ps1[:], usc[:, b:b + 1])
            # m = 1/(1 + exp(-10*arg)), arg = fl(fl(s*u) - fl(thr*u)) --
            # matches XLA's f32 sigmoid incl. the saturation-to-1.0 cliff
            # (the HW Sigmoid table is 1 ulp off there, which flips
            # (1-m)*(-1e9) between 0 and -59.6 and corrupts half the rows).
            arg = epool.tile([128, TDS], F32, tag="arg")
            nc.vector.tensor_scalar(arg[:, 0:nd], ps[:, 0:nd], usc[:, b:b + 1],
                                    thru[:], op0=ALU.mult, op1=ALU.subtract)
            z = epool.tile([128, TDS], F32, tag="z")
            nc.scalar.activation(z[:, 0:nd], arg[:, 0:nd], ACTF.Exp, scale=-10.0)
            nc.gpsimd.tensor_scalar(z[:, 0:nd], z[:, 0:nd], 1.0, None, op0=ALU.add)
            e = epool.tile([128, TDS], F32, tag="e")
            nc.vector.reciprocal(e[:, 0:nd], z[:, 0:nd])
            f = fpool.tile([128, TDS], F32, tag="f")
            # W = fl(1 - m) = fl(-m) + 1 (exact negate), F = fl(W * -1e9)
            nc.gpsimd.tensor_scalar(f[:, 0:nd], e[:, 0:nd], -1.0, 1.0,
                                    op0=ALU.mult, op1=ALU.add)
            nc.gpsimd.tensor_scalar(f[:, 0:nd], f[:, 0:nd], NEG, None,
                                    op0=ALU.mult)
            amin = aminpool.tile([128, TDS], F32, tag="amin")
            off = 496 - 16 * kb
            nc.vector.tensor_tensor(amin[:, 0:nd], f[:, 0:nd],
                                    cm3[:, off:off + nd], op=ALU.min)
            negm = smallpool.tile([128, 1], F32, tag="negm")
            nc.vector.tensor_reduce(negm[:], amin[:, 0:nd], axis=AX.X,
                                    op=ALU.max, negate=True)
            # per-row constant value of the suffix (fully-masked) columns:
            # C = exp(fl(-1e9 - M)); 0 for healthy rows, the uniform weight
            # for desperate rows.
            b2 = tinypool.tile([128, 1], F32, tag="b2")
            nc.vector.tensor_scalar(b2[:], negm[:], NEG, None, op0=ALU.add)
            cee = smallpool.tile([128, 1], F32, tag="cee")
            nc.scalar.activation(cee[:], b2[:], ACTF.Exp)
            amins[b] = amin
            negms[b] = negm
            cees[b] = cee
            # C^T for the rank-1 suffix inject
            if kb + 1 <= CH - 1:
                ctp = swppool.tile([1, 128], F32, tag="swp")
                nc.tensor.transpose(ctp[:], cee[:], ident[:])
                nc.scalar.copy(call[0:1, 128 * b:128 * b + 128], ctp[:])
            if dbg is not None and b == 0:
                nc.sync.dma_start(out=dbg["E0"], in_=e[:])
                nc.sync.dma_start(out=dbg["F0"], in_=f[:])
                nc.sync.dma_start(out=dbg["AM0"], in_=amin[:])

        def score_block(b):
            kb = KBS[r][b]
            ncol = 128 * (kb + 1)
            nsuf = T - ncol
            n512 = (ncol + 511) // 512
            p = ppool.tile([128, T], F32, tag="p")
            ptiles[b] = p
            for j in range(n512):
                lim = min(512, ncol - 512 * j)
                ps = ps2pool.tile([128, 512], F32, tag="ps2")
                nc.tensor.matmul(
                    ps[:, 0:lim],
                    qt[:, 128 * b:128 * b + 128],
                    kt[:, 512 * j:512 * j + lim],
                )
                # X = fl(fl(S*scale) + A), A = min(F, ds-causal) broadcast x8
                a_sl = amins[b][:, 64 * j:64 * j + lim // 8].unsqueeze(-1) \
                    .to_broadcast([128, lim // 8, STRIDE])
                x_v = p[:, 512 * j:512 * j + lim].rearrange(
                    "p (c s) -> p c s", s=STRIDE)
                nc.vector.scalar_tensor_tensor(
                    out=x_v, in0=ps[:, 0:lim].rearrange("p (c s) -> p c s",
                                                        s=STRIDE),
                    scalar=SCALE, in1=a_sl, op0=ALU.mult, op1=ALU.add)
                if 512 * j <= 128 * kb < 512 * j + lim:
                    dl = 128 * kb - 512 * j
                    tmp = tmppool.tile([128, 128], F32, tag="tmp")
                    nc.scalar.activation(tmp[:], ps[:, dl:dl + 128],
                                         ACTF.Identity, bias=negc[:], scale=SCALE)
                    nc.vector.copy_predicated(
                        p[:, 128 * kb:128 * kb + 128], c01inv[:], tmp[:])
            if dbg is not None and b == 0:
                nc.sync.dma_start(out=dbg["X0"], in_=p[:])
            ssum = tinypool.tile([128, 1], F32, tag="ssum")
            nc.scalar.activation(p[:, 0:ncol], p[:, 0:ncol], ACTF.Exp,
                                 bias=negms[b][:], scale=1.0, accum_out=ssum[:])
            if dbg is not None and b == 0:
                nc.sync.dma_start(out=dbg["P0"], in_=p[:])
                nc.sync.dma_start(out=dbg["NM0"], in_=negms[b][:])
                nc.sync.dma_start(out=dbg["SS0"], in_=ssum[:])
            # denominator = mainsum + C * n_suffix
            den = tinypool.tile([128, 1], F32, tag="den")
            nc.vector.scalar_tensor_tensor(
                out=den[:], in0=cees[b][:], scalar=float(nsuf), in1=ssum[:],
                op0=ALU.mult, op1=ALU.add)
            rsum = smallpool.tile([128, 1], F32, tag="rsum")
            nc.vector.reciprocal(rsum[:], den[:])
            rsums[b] = rsum

        osb = inpool.tile([128, NB * 128], F32, tag="osb")
        onesr = cpool.tile([128, 1], F32R, tag="onesr")
        onesf = cpool.tile([128, 1], F32, tag="onesf")
        nc.gpsimd.memset(onesf[:], 1.0)
        nc.vector.tensor_copy(onesr[:], onesf[:])
        swall = swsbpool.tile([1, NB * 128], F32R, tag="swall")
        call = swsbpool.tile([1, NB * 128], F32R, tag="call")

        # ---- suffix V column-sums SW(cb) = sum_{c >= cb} V[c-chunk],
        # stored as [1, 128]-slices of a partition-0 tile ----
        swp = swppool.tile([1, 128], F32, tag="swp")
        emitted = 0
        prev = CH
        for b in range(NB - 1, -1, -1):
            cb = KBS[r][b] + 1
            for c in range(cb, prev):
                emitted += 1
                # stop before each snapshot read (sim requirement); the psum
                # keeps accumulating across groups via start=False.
                nc.tensor.matmul(swp[:], onesr[:], vsb[:, 128 * c:128 * c + 128],
                                 start=(emitted == 1), stop=(c == cb),
                                 skip_group_check=(emitted != 1))
            prev = cb
            if cb <= CH - 1:
                nc.scalar.copy(swall[0:1, 128 * b:128 * b + 128], swp[:])

        # ---- interleaved main loop: per group, run the 4 blocks' mask
        # chain + scores + exp, then the group's PV.  Chunk c feeds only
        # blocks with kb >= c; the suffix columns contribute the rank-1
        # term SW(kb+1) (x) C, injected directly into the PSUM accumulator.
        def pv_emit(g, jlo, jhi, outt):
            """PV accumulation for blocks 4g+jlo .. 4g+jhi into outt columns
            [128*jlo, 128*(jhi+1)); suffix rank-1 injects included."""
            kbs = [KBS[r][4 * g + j] for j in range(jlo, jhi + 1)]
            cmax = kbs[-1]
            for c in range(cmax + 1):
                jmin = jlo + min(i for i, kb in enumerate(kbs) if kb >= c)
                hi = 128 * (jhi + 1)
                ptp = ptpool.tile([128, 512], F32, tag="ptp")
                for j in range(jmin, jhi + 1):
                    nc.tensor.transpose(
                        ptp[:, 128 * j:128 * j + 128],
                        ptiles[4 * g + j][:, 128 * c:128 * c + 128],
                        ident[:],
                    )
                pts = ptsbpool.tile([128, 512], F32R, tag="pts")
                mod = 2 if jlo != 0 else 8
                if c % mod == 0:
                    nc.vector.tensor_copy(pts[:, 128 * jmin:hi],
                                          ptp[:, 128 * jmin:hi])
                else:
                    nc.scalar.copy(pts[:, 128 * jmin:hi],
                                   ptp[:, 128 * jmin:hi])
                nc.tensor.matmul(
                    outt[:, 128 * jmin:hi],
                    vsb[:, 128 * c:128 * c + 128],
                    pts[:, 128 * jmin:hi],
                    start=(c == 0), stop=(c == cmax),
                    skip_group_check=(jlo != 0),
                )
            for j in range(jlo, jhi + 1):
                b = 4 * g + j
                if KBS[r][b] + 1 <= CH - 1:
                    nc.tensor.matmul(
                        outt[:, 128 * j:128 * j + 128],
                        swall[0:1, 128 * b:128 * b + 128],
                        call[0:1, 128 * b:128 * b + 128],
                        start=False, stop=True, skip_group_check=True,
                    )

        def pv_epilogue(g, outt):
            outt_sb = outtsbpool.tile([128, 512], F32, tag="outtsb")
            nc.any.tensor_copy(outt_sb[:], outt[:])
            ops = ptpool.tile([128, 512], F32, tag="ptp")
            for j in range(4):
                nc.tensor.transpose(
                    ops[:, 128 * j:128 * j + 128],
                    outt_sb[:, 128 * j:128 * j + 128], ident[:])
            for j in range(4):
                b = 4 * g + j
                nc.scalar.mul(osb[:, 128 * b:128 * b + 128],
                              ops[:, 128 * j:128 * j + 128], rsums[b][:])
                row0 = 128 * b
                nc.sync.dma_start(out=Od[row0:row0 + 128, :],
                                  in_=osb[:, 128 * b:128 * b + 128])

        for gi, g in enumerate(GORDER):
            last = gi >= len(GORDER) - 2
            outt = outtpool.tile([128, 512], F32, tag="outt")
            if last:
                # split the final group's PV so its first half overlaps the
                # last two blocks' scores/exp (shortens the serial tail)
                for j in range(2):
                    mask_chain(4 * g + j)
                    score_block(4 * g + j)
                pv_emit(g, 0, 1, outt)
                for j in range(2, 4):
                    mask_chain(4 * g + j)
                    score_block(4 * g + j)
                pv_emit(g, 2, 3, outt)
            else:
                for j in range(4):
                    mask_chain(4 * g + j)
                    score_block(4 * g + j)
                pv_emit(g, 0, 3, outt)
            pv_epilogue(g, outt)


_PROGRAMS = {}


def build_program(r: int, debug=False):
    key = (r, debug)
    if key in _PROGRAMS:
        return _PROGRAMS[key]
    nc = bacc.Bacc("TRN2", target_bir_lowering=False, debug=False)
    Qd = nc.dram_tensor("Q", [NLQ, HD], F32, kind="ExternalInput").ap()
    Kd = nc.dram_tensor("K", [T, HD], F32, kind="ExternalInput").ap()
    Vd = nc.dram_tensor("V", [T, HD], F32R, kind="ExternalInput").ap()
    Ud = nc.dram_tensor("UBT", [128, NB], F32, kind="ExternalInput").ap()
    Od = nc.dram_tensor("OUT", [NLQ, HD], F32, kind="ExternalOutput").ap()
    dbg = None
    if debug:
        dbg = {}
        for nm, shp in (("SDS0", [128, TDS + 1]), ("SDS1", [128, TDS + 1]),
                        ("E0", [128, TDS]),
                        ("F0", [128, TDS]), ("AM0", [128, TDS]),
                        ("X0", [128, T]), ("P0", [128, T]),
                        ("NM0", [128, 1]), ("SS0", [128, 1])):
            dbg[nm] = nc.dram_tensor(nm, shp, F32, kind="ExternalOutput").ap()
    with tile.TileContext(nc) as tc:
        _kernel_body(tc, r, Qd, Kd, Vd, Ud, Od, dbg)
    nc.compile()
    _PROGRAMS[key] = nc
    return nc


def shard_inputs(Q, K, V, U):
    """Return per-core input dicts: core = 4*r + h (devices 0-3 parity 0)."""
    maps = []
    Qr = Q[0].reshape(NH, 2 * NB, QPB, HD)
    Ur = U[0].reshape(2 * NB, QPB)
    for r in range(2):
        for h in range(NH):
            qsh = np.ascontiguousarray(Qr[h, KBS[r]].reshape(NLQ, HD))
            ubt = np.ascontiguousarray(Ur[KBS[r]].T)        # [128, NB]
            maps.append({
                "Q": qsh,
                "K": np.ascontiguousarray(K[0, h]),
                "V": np.ascontiguousarray(V[0, h]),
                "UBT": ubt,
            })
    return maps


def unshard_output(outs):
    """outs: list of 8 dicts with 'OUT' [2048, 128] in core order above."""
    O = np.empty((B, NH, T, HD), np.float32)
    Ov = O[0].reshape(NH, 2 * NB, QPB, HD)
    i = 0
    for r in range(2):
        for h in range(NH):
            Ov[h, KBS[r]] = outs[i]["OUT"].reshape(NB, QPB, HD)
            i += 1
    return O


def _run_concurrent(in_maps):
    """Dispatch parity-0 program on devices 0-3 and parity-1 on devices 4-7,
    concurrently (adapted from concourse.bass2jax.run_bass_via_pjrt)."""
    import jax
    from jax.sharding import Mesh, PartitionSpec
    from jax.experimental.shard_map import shard_map
    from concourse import bass2jax

    bass2jax.install_neuronx_cc_hook()
    devices = jax.devices()
    assert len(devices) >= 8, f"need 8 neuron cores, got {len(devices)}"

    pending = []
    for r in range(2):
        nc = build_program(r)
        maps = in_maps[4 * r:4 * r + 4]
        pname = nc.partition_id_tensor.name if nc.partition_id_tensor else None
        in_names, out_names, out_avals, zero_outs = [], [], [], []
        for alloc in nc.m.functions[0].allocations:
            if not isinstance(alloc, mybir.MemoryLocationSet):
                continue
            name = alloc.memorylocations[0].name
            if alloc.kind == "ExternalInput":
                if name != pname:
                    in_names.append(name)
            elif alloc.kind == "ExternalOutput":
                out_names.append(name)
                shape = tuple(alloc.tensor_shape)
                dtype = mybir.dt.np(alloc.dtype)
                out_avals.append(jax.core.ShapedArray(shape, dtype))
                zero_outs.append(np.zeros(shape, dtype))
        n_params = len(in_names)
        n_outs = len(out_avals)
        all_names = in_names + out_names
        if pname is not None:
            all_names = all_names + [pname]
        donate = tuple(range(n_params, n_params + n_outs))

        def _body(*args, _nc=nc, _avals=tuple(out_avals),
                  _names=tuple(all_names), _onames=tuple(out_names),
                  _pname=pname):
            operands = list(args)
            if _pname is not None:
                operands.append(bass2jax.partition_id_tensor())
            outs = bass2jax._bass_exec_p.bind(
                *operands,
                out_avals=_avals,
                in_names=_names,
                out_names=_onames,
                lowering_input_output_aliases=(),
                sim_require_finite=True,
                sim_require_nnan=True,
                nc=_nc,
            )
            return tuple(outs)

        mesh = Mesh(np.asarray(devices[4 * r:4 * r + 4]), ("core",))
        in_specs = (PartitionSpec("core"),) * (n_params + n_outs)
        out_specs = (PartitionSpec("core"),) * n_outs
        fn = jax.jit(
            shard_map(_body, mesh=mesh, in_specs=in_specs,
                      out_specs=out_specs, check_rep=False),
            donate_argnums=donate, keep_unused=True,
        )
        per_core = [[np.asarray(m[nm]) for nm in in_names] for m in maps]
        concat_in = [
            np.concatenate([per_core[c][i] for c in range(4)], axis=0)
            for i in range(n_params)
        ]
        concat_zero = [
            np.concatenate([z] * 4, axis=0) for z in zero_outs
        ]
        out_arrs = fn(*concat_in, *concat_zero)   # async dispatch
        pending.append((out_arrs, out_names))

    results = []
    for r, (out_arrs, out_names) in enumerate(pending):
        outs = [np.asarray(a) for a in out_arrs]   # force
        for c in range(4):
            d = {}
            for i, nm in enumerate(out_names):
                n0 = outs[i].shape[0] // 4
                d[nm] = outs[i][c * n0:(c + 1) * n0]
            results.append(d)
    return results


def kernel(**inputs):
    Q = np.asarray(inputs["Q"], np.float32)
    K = np.asarray(inputs["K"], np.float32)
    V = np.asarray(inputs["V"], np.float32)
    U = np.asarray(inputs["U"], np.float32)
    in_maps = shard_inputs(Q, K, V, U)
    results = _run_concurrent(in_maps)
    return unshard_output(results)

